# revision 30
# baseline (speedup 1.0000x reference)
"""Trainium2 Bass kernel for the ConfusionClassification criterion.

Computes, for full inputs
    pred_logits      [64, 65536, 2] f32
    pred_confusion   [64, 65536, 4] f32
    target_classes   [64, 65536]    int64 (values 0/1)
the scalar loss
    src  = argmax(pred_logits, -1)
    c    = g==1 ? (src==1 ? 1 : 2) : (src==1 ? 3 : 0)
    loss = mean_{b,n}( logsumexp(pred_confusion) - pred_confusion[c] )

Sharding: pure data-parallel over the batch dim; each of the 8 NeuronCores
processes 8 batches (524288 points).  Each core emits per-partition partial
sums of logsumexp and of the selected logit; the host reduces them.

Per-core device pipeline (per tile of 128 x L points):
  DMA   : conf [128,4L], logits [128,2L], target-as-f32 [128,L]
  ACT   : e = exp(conf)                      (4L elems)
  DVE   : s = (e0+e1)+(e2+e3)                (3 tensor_tensor adds)
  DVE   : p = is_gt(l1, l0)                  (argmax as mask)
  DVE   : 3x copy_predicated, in-place on the conf tile:
            X0 <- X3 where p ; X2 <- X1 where p ; X0 <- X2 where g
          leaving X0 = pred_confusion[c]
  ACT   : Ln(s)   with accum_out -> sum(lse)  per partition
  ACT   : Copy(X0) with accum_out -> sum(sel) per partition
"""

import sys

for _p in ("/opt/trn_rl_repo",):
    if _p not in sys.path:
        sys.path.insert(0, _p)

import numpy as np

import concourse.bacc as bacc
import concourse.bass as bass
import concourse.mybir as mybir
from concourse.bass_utils import run_bass_kernel_spmd
from concourse.mybir import AluOpType
from concourse.tile import TileContext

AF = mybir.ActivationFunctionType
F32 = mybir.dt.float32
U8 = mybir.dt.uint8
I8 = mybir.dt.int8

P = 128
B, N = 64, 65536
M = 8                      # cores
BS = B // M                # batches per core
NP_CORE = BS * N           # points per core
T = 4                      # tiles per core
L = NP_CORE // (T * P)     # points per partition per tile


def emit_loss_kernel(
    nc, conf, lg, tgt, out_acc, n_tiles, width, io_bufs=2, tmp_bufs=2, repeat=1,
    ops=frozenset({"exp", "sums", "pm", "sel", "ln", "selsum"}),
):
    """Emit the per-core loss kernel.

    conf: DRAM AP [T, 128, 4L]  interleaved 4-class confusion logits
    lg:   DRAM AP [T, 128, 2L]  interleaved 2-class prediction logits
    tgt:  DRAM AP [T, 128, L]   target class as uint8 0/1
    out_acc: DRAM AP [128, 2T]  cols [0,T) = sum(lse), cols [T,2T) = sum(sel)
    repeat: re-run the whole pass this many times (benchmark differencing)
    """
    with TileContext(nc) as tc:
        with (
            tc.tile_pool(name="io", bufs=io_bufs) as io_pool,
            tc.tile_pool(name="tmp", bufs=tmp_bufs) as tmp_pool,
            tc.tile_pool(name="accp", bufs=1) as acc_pool,
        ):
            acc = acc_pool.tile([P, 2 * n_tiles], F32)
            if not ({"ln", "selsum"} & ops):
                nc.vector.memset(acc[:], 0.0)
            for _rep, t in ((r, t) for r in range(repeat) for t in range(n_tiles)):
                conf_t = io_pool.tile([P, 4 * width], F32, tag="conf")
                lg_t = io_pool.tile([P, 2 * width], F32, tag="lg")
                tgt_t = io_pool.tile([P, width], U8, tag="tgt")
                nc.sync.dma_start(out=conf_t[:], in_=conf[t])
                nc.sync.dma_start(out=lg_t[:], in_=lg[t])
                nc.sync.dma_start(out=tgt_t[:], in_=tgt[t])

                e_t = tmp_pool.tile([P, 4 * width], F32, tag="e")
                t01 = tmp_pool.tile([P, width], F32, tag="t01")
                t23 = tmp_pool.tile([P, width], F32, tag="t23")
                s = tmp_pool.tile([P, width], F32, tag="s")
                pm = tmp_pool.tile([P, width], I8, tag="pm")

                X = conf_t[:].rearrange("p (l k) -> p l k", k=4)
                E = e_t[:].rearrange("p (l k) -> p l k", k=4)
                LG = lg_t[:].rearrange("p (l k) -> p l k", k=2)

                if "exp" in ops:
                    nc.scalar.activation(e_t[:], conf_t[:], AF.Exp)
                if "sums" in ops:
                    nc.vector.tensor_tensor(t01[:], E[:, :, 0], E[:, :, 1], AluOpType.add)
                    nc.vector.tensor_tensor(t23[:], E[:, :, 2], E[:, :, 3], AluOpType.add)
                    nc.vector.tensor_tensor(s[:], t01[:], t23[:], AluOpType.add)
                if "pm" in ops:
                    nc.vector.tensor_tensor(pm[:], LG[:, :, 1], LG[:, :, 0], AluOpType.is_gt)
                if "sel" in ops:
                    nc.vector.copy_predicated(X[:, :, 0], pm[:], X[:, :, 3])
                    nc.vector.copy_predicated(X[:, :, 2], pm[:], X[:, :, 1])
                    nc.vector.copy_predicated(X[:, :, 0], tgt_t[:], X[:, :, 2])
                if "ln" in ops:
                    nc.scalar.activation(
                        t01[:], s[:], AF.Ln, accum_out=acc[:, t : t + 1]
                    )
                if "selsum" in ops:
                    nc.scalar.activation(
                        t23[:], X[:, :, 0], AF.Copy,
                        accum_out=acc[:, n_tiles + t : n_tiles + t + 1],
                    )
            nc.sync.dma_start(out=out_acc, in_=acc[:])
    return nc


def _pin_act_table_set(nc, set_id):
    """Replace the alternating per-function ACT table loads with a single
    load of one set that contains every function the kernel uses.

    bacc's insert_act_table_loads greedily picks the first act_info.json set
    containing each function, so an Exp/Ln/Copy mix thrashes between
    exp_and_others and natural_log -- ~2.7us per reload, serialized on ACT.
    natural_log_exp_and_others (set 6 on gen3) holds Exp, Ln and Copy, so one
    load suffices.  The inserted loads carry no sync_info, so dropping the
    extras cannot break semaphore bookkeeping.
    """
    for fn in nc.m.functions:
        for blk in fn.blocks:
            first = True
            keep = []
            for ins in blk.instructions:
                if isinstance(ins, mybir.InstLoadActFuncSet):
                    assert ins.sync_info is None or (
                        not ins.sync_info.on_wait and not ins.sync_info.on_update
                    )
                    if not first:
                        continue
                    ins.act_func_set_id = set_id
                    first = False
                keep.append(ins)
            if len(keep) != len(blk.instructions):
                blk.instructions[:] = keep


def build_nc(n_tiles=T, width=L, io_bufs=2, tmp_bufs=2, repeat=1,
             ops=frozenset({"exp", "sums", "pm", "sel", "ln", "selsum"})):
    nc = bacc.Bacc("TRN2", target_bir_lowering=False, debug=False)
    conf = nc.dram_tensor("conf", [n_tiles, P, 4 * width], F32, kind="ExternalInput").ap()
    lg = nc.dram_tensor("lg", [n_tiles, P, 2 * width], F32, kind="ExternalInput").ap()
    tgt = nc.dram_tensor("tgt", [n_tiles, P, width], U8, kind="ExternalInput").ap()
    out_acc = nc.dram_tensor("acc", [P, 2 * n_tiles], F32, kind="ExternalOutput").ap()
    emit_loss_kernel(
        nc, conf, lg, tgt, out_acc, n_tiles, width, io_bufs, tmp_bufs, repeat, ops
    )
    nc.finalize()
    _pin_act_table_set(nc, 6)
    return nc


BF16 = mybir.dt.bfloat16
I16 = mybir.dt.int16


def emit_loss_kernel_v2(
    nc, conf, lg, tgt, out_acc, n_tiles, width, io_bufs=3, tmp_bufs=2, repeat=1,
    ops=frozenset({"exp", "sums", "pm", "sel", "ln", "selsum"}),
):
    """Planar bf16 variant.

    conf: DRAM AP [T, 128, 4L] bf16, free dim = [class k][point l] (planar)
    lg:   DRAM AP [T, 128, 2L] bf16 planar
    tgt:  DRAM AP [T, 128, L]  uint8
    out_acc: DRAM AP [128, T+1]: cols [0,T) per-partition sum(lse); entry
      [0, T] = total sum(sel) (from the PE column-sum path, partition 0 only).

    All DVE ops are step-1 16-bit -> 2x_1P mode.  sum(sel) is computed by the
    TensorEngine as a ones-vector column sum accumulated in PSUM across tiles,
    then one ACT Copy+accum over [1, L] at the end.
    """
    W = width
    with TileContext(nc) as tc:
        with (
            tc.tile_pool(name="io", bufs=io_bufs) as io_pool,
            tc.tile_pool(name="tmp", bufs=tmp_bufs) as tmp_pool,
            tc.tile_pool(name="accp", bufs=1) as acc_pool,
            tc.tile_pool(name="psum", bufs=1, space="PSUM") as psum_pool,
        ):
            acc = acc_pool.tile([P, 2 * n_tiles + 1], F32)
            ones = acc_pool.tile([P, 1], BF16)
            nc.vector.memset(ones[:], 1.0)
            nc.vector.memset(acc[:], 0.0)
            selp = psum_pool.tile([1, W], F32)
            n_chunks = (W + 511) // 512
            total = repeat * n_tiles
            it = 0
            for _rep, t in ((r, t) for r in range(repeat) for t in range(n_tiles)):
                conf_t = io_pool.tile([P, 4 * W], BF16, tag="conf")
                lg_t = io_pool.tile([P, 2 * W], BF16, tag="lg")
                tgt_t = io_pool.tile([P, W], U8, tag="tgt")
                nc.sync.dma_start(out=conf_t[:], in_=conf[t])
                nc.sync.dma_start(out=lg_t[:], in_=lg[t])
                nc.sync.dma_start(out=tgt_t[:], in_=tgt[t])

                e_t = tmp_pool.tile([P, 4 * W], BF16, tag="e")
                s01 = tmp_pool.tile([P, W], BF16, tag="s01")
                s23 = tmp_pool.tile([P, W], BF16, tag="s23")
                s = tmp_pool.tile([P, W], BF16, tag="s")
                pm = tmp_pool.tile([P, W], I16, tag="pm")

                ca = conf_t[:]
                ea = e_t[:]
                la = lg_t[:]
                Xs = [ca[:, k * W : (k + 1) * W] for k in range(4)]
                Es = [ea[:, k * W : (k + 1) * W] for k in range(4)]
                L0 = la[:, 0:W]
                L1 = la[:, W : 2 * W]

                if "pm" in ops:
                    nc.vector.tensor_tensor(pm[:], L1, L0, AluOpType.is_gt)
                if "exp" in ops:
                    nc.scalar.activation(e_t[:], conf_t[:], AF.Exp)
                if "sums" in ops:
                    eng01 = nc.gpsimd if "pooladds" in ops else nc.vector
                    eng01.tensor_tensor(s01[:], Es[0], Es[1], AluOpType.add)
                    eng01.tensor_tensor(s23[:], Es[2], Es[3], AluOpType.add)
                    nc.vector.tensor_tensor(s[:], s01[:], s23[:], AluOpType.add)
                if "sel" in ops:
                    nc.vector.copy_predicated(Xs[0], pm[:], Xs[3])
                    nc.vector.copy_predicated(Xs[2], pm[:], Xs[1])
                    nc.vector.copy_predicated(Xs[0], tgt_t[:], Xs[2])
                if "ln" in ops:
                    nc.scalar.activation(
                        s01[:], s[:], AF.Ln, accum_out=acc[:, t : t + 1]
                    )
                if "selsum" in ops:
                    if "selsum_pe" in ops:
                        for c in range(n_chunks):
                            lo, hi = c * 512, min((c + 1) * 512, W)
                            nc.tensor.matmul(
                                selp[:, lo:hi],
                                ones[:],
                                Xs[0][:, lo:hi],
                                start=(it == 0),
                                stop=(it == total - 1),
                            )
                    else:
                        nc.scalar.activation(
                            s23[:], Xs[0], AF.Copy,
                            accum_out=acc[:, n_tiles + 1 + t : n_tiles + 2 + t],
                        )
                it += 1
            if "selsum" in ops and "selsum_pe" in ops:
                seljunk = acc_pool.tile([1, W], F32)
                nc.scalar.activation(
                    seljunk[:],
                    selp[:, :],
                    AF.Copy,
                    accum_out=acc[0:1, n_tiles : n_tiles + 1],
                )
            nc.sync.dma_start(out=out_acc, in_=acc[:])
    return nc


def build_nc_v2(n_tiles=T, width=None, io_bufs=3, tmp_bufs=2, repeat=1,
                ops=frozenset({"exp", "sums", "pm", "sel", "ln", "selsum"})):
    if width is None:
        width = NP_CORE // (n_tiles * P)
    nc = bacc.Bacc("TRN2", target_bir_lowering=False, debug=False)
    conf = nc.dram_tensor("conf", [n_tiles, P, 4 * width], BF16, kind="ExternalInput").ap()
    lg = nc.dram_tensor("lg", [n_tiles, P, 2 * width], BF16, kind="ExternalInput").ap()
    tgt = nc.dram_tensor("tgt", [n_tiles, P, width], U8, kind="ExternalInput").ap()
    out_acc = nc.dram_tensor("acc", [P, 2 * n_tiles + 1], F32, kind="ExternalOutput").ap()
    emit_loss_kernel_v2(
        nc, conf, lg, tgt, out_acc, n_tiles, width, io_bufs, tmp_bufs, repeat, ops
    )
    nc.finalize()
    _pin_act_table_set(nc, 6)
    return nc


def shard_inputs_v2(pred_logits, pred_confusion, target_classes, n_tiles=T, width=None):
    import ml_dtypes

    if width is None:
        width = NP_CORE // (n_tiles * P)
    T_, L_ = n_tiles, width
    bf16 = ml_dtypes.bfloat16
    in_maps = []
    for i in range(M):
        sl = slice(i * BS, (i + 1) * BS)
        conf = (
            np.asarray(pred_confusion[sl], dtype=np.float32)
            .reshape(T_, P, L_, 4)
            .transpose(0, 1, 3, 2)
            .astype(bf16)
            .reshape(T_, P, 4 * L_)
        )
        conf = np.ascontiguousarray(conf)
        lg = (
            np.asarray(pred_logits[sl], dtype=np.float32)
            .reshape(T_, P, L_, 2)
            .transpose(0, 1, 3, 2)
            .astype(bf16)
            .reshape(T_, P, 2 * L_)
        )
        lg = np.ascontiguousarray(lg)
        tgt = np.asarray(target_classes[sl], dtype=np.uint8).reshape(T_, P, L_)
        in_maps.append({"conf": conf, "lg": lg, "tgt": tgt})
    return in_maps


def shard_inputs(pred_logits, pred_confusion, target_classes):
    in_maps = []
    for i in range(M):
        sl = slice(i * BS, (i + 1) * BS)
        conf = np.ascontiguousarray(pred_confusion[sl], dtype=np.float32).reshape(
            T, P, 4 * L
        )
        lg = np.ascontiguousarray(pred_logits[sl], dtype=np.float32).reshape(
            T, P, 2 * L
        )
        tgt = np.asarray(target_classes[sl], dtype=np.uint8).reshape(T, P, L)
        in_maps.append({"conf": conf, "lg": lg, "tgt": tgt})
    return in_maps


F8 = mybir.dt.float8e4


V3_ALL_OPS = frozenset({"exp", "lnacc", "selstt", "suma"})


def emit_loss_kernel_v3(nc, conf, lg, out_acc, n_tiles, width, io_bufs=3, tmp_bufs=3,
                        use_pe=True, ops=V3_ALL_OPS):
    """fp8 planar variant with host-permuted confusion planes.

    conf: DRAM AP [T, 128, 4W] fp8, planes [A|B|C|D]:
      A = conf logit of the class selected when argmax(pred)=0,
      B = class selected when argmax(pred)=1, C/D = the other two classes
      (per-point permutation applied on host from target_classes).
    lg:  DRAM AP [T, 128, 2W] fp8, planes [L0|L1].
    out_acc: [128, 2T+1] f32: cols [0,T) = per-partition sum(ln s);
      cols [T,2T) = per-partition sum(p*(B-A)); acc[0, 2T] = total sum(A).

    loss*B*N = sum(ln s) - sum(A) - sum(p*(B-A))

    Engine split per tile: ACT exp[4W]+ln[W]; DVE dz, dl, s2, s;
    Pool fused (dl>0)*dz with accumulate; PE ones-matmul column-sums of A
    accumulated in PSUM across tiles.
    """
    W = width
    T = n_tiles
    with TileContext(nc) as tc:
        with (
            tc.tile_pool(name="io", bufs=io_bufs) as io_pool,
            tc.tile_pool(name="tmp", bufs=tmp_bufs) as tmp_pool,
            tc.tile_pool(name="accp", bufs=1) as acc_pool,
            tc.tile_pool(name="psum", bufs=1, space="PSUM") as psum_pool,
        ):
            acc = acc_pool.tile([P, 2 * T + 1 + (0 if use_pe else n_tiles)], F32)
            if use_pe and "suma" in ops:
                ones = acc_pool.tile([P, 1], F8)
                nc.vector.memset(ones[:], 1.0)
            if ops == V3_ALL_OPS:
                nc.vector.memset(acc[:, 2 * T : 2 * T + 1], 0.0)
            else:
                nc.vector.memset(acc[:], 0.0)
            if use_pe and "suma" in ops:
                psumA = psum_pool.tile([1, W], F32)
            else:
                psumA = None
            n_chunks = (W + 511) // 512

            tiles = []
            for t in range(T):
                conf_t = io_pool.tile([P, 4 * W], F8, tag="conf")
                lg_t = io_pool.tile([P, 2 * W], F8, tag="lg")
                nc.sync.dma_start(out=conf_t[:], in_=conf[t])
                nc.sync.dma_start(out=lg_t[:], in_=lg[t])

                e_t = tmp_pool.tile([P, 4 * W], BF16, tag="e")
                s2 = tmp_pool.tile([P, 2 * W], BF16, tag="s2")
                s = tmp_pool.tile([P, W], BF16, tag="s")
                dz = tmp_pool.tile([P, W], BF16, tag="dz")
                pm = tmp_pool.tile([P, W], I16, tag="pm")
                junkp = tmp_pool.tile([P, W], BF16, tag="junkp")

                ca = conf_t[:]
                A = ca[:, 0:W]
                Bp = ca[:, W : 2 * W]
                la = lg_t[:]

                # ACT: exp over all four planes (software-pipelined: ln of
                # the previous tile is emitted after this exp).
                if "exp" in ops:
                    nc.scalar.activation(e_t[:], ca, AF.Exp)
                if "lnacc" in ops and tiles:
                    (ps, pjl, pt) = tiles[-1]
                    nc.scalar.activation(
                        pjl[:], ps[:], AF.Ln, accum_out=acc[:, pt : pt + 1]
                    )

                # DVE
                if {"sel", "selsub", "selstt"} & ops:
                    nc.vector.tensor_tensor(dz[:], Bp, A, AluOpType.subtract)
                if "selstt" in ops:
                    dl = tmp_pool.tile([P, W], BF16, tag="dl")
                    nc.vector.tensor_tensor(
                        dl[:], la[:, W : 2 * W], la[:, 0:W], AluOpType.subtract
                    )
                    # fused: junkp = (dl > 0) * dz; accum = per-partition sum
                    nc.vector.scalar_tensor_tensor(
                        junkp[:], dl[:], 0.0, dz[:],
                        AluOpType.is_gt, AluOpType.mult,
                        accum_out=acc[:, T + t : T + t + 1],
                    )
                elif {"sel", "selsub"} & ops:
                    nc.vector.tensor_tensor(
                        pm[:], la[:, W : 2 * W], la[:, 0:W], AluOpType.is_gt
                    )
                ea = e_t[:]
                if "lnacc" in ops:
                    nc.vector.tensor_tensor(
                        s2[:], ea[:, 0 : 2 * W], ea[:, 2 * W : 4 * W], AluOpType.add
                    )
                    nc.vector.tensor_tensor(
                        s[:], s2[:, 0:W], s2[:, W : 2 * W], AluOpType.add
                    )

                if {"sel", "selttr"} & ops:
                    # DVE fused: sum over free dim of pm * dz
                    nc.vector.tensor_tensor_reduce(
                        junkp[:], pm[:], dz[:], 1.0, 0.0,
                        AluOpType.mult, AluOpType.add,
                        accum_out=acc[:, T + t : T + t + 1],
                    )
                elif "selmul" in ops:
                    # DVE: pm*dz at 2x, then free-dim sum at 4x
                    junkq = tmp_pool.tile([P, W], BF16, tag="junkq")
                    nc.vector.tensor_tensor(
                        junkp[:], pm[:], dz[:], AluOpType.mult
                    )
                    nc.vector.tensor_scalar(
                        junkq[:], junkp[:], 1.0, 1.0,
                        AluOpType.mult, AluOpType.mult,
                        accum_out=acc[:, T + t : T + t + 1],
                    )

                if "suma" not in ops:
                    pass
                elif use_pe:
                    # PE: column sums of the A plane, accumulated across tiles
                    for c in range(n_chunks):
                        lo, hi = c * 512, min((c + 1) * 512, W)
                        nc.tensor.matmul(
                            psumA[:, lo:hi], ones[:], A[:, lo:hi],
                            start=(t == 0), stop=(t == T - 1),
                        )
                else:
                    # DVE: per-partition sum of the A plane
                    junkA_t = tmp_pool.tile([P, W], BF16, tag="junkA")
                    nc.vector.tensor_scalar(
                        junkA_t[:], A, 1.0, 1.0, AluOpType.mult, AluOpType.mult,
                        accum_out=acc[:, 2 * T + 1 + t : 2 * T + 2 + t],
                    )

                junkl = tmp_pool.tile([P, W], BF16, tag="junkl")
                tiles.append((s, junkl, t))

            if "lnacc" in ops:
                (ps, pjl, pt) = tiles[-1]
                nc.scalar.activation(
                    pjl[:], ps[:], AF.Ln, accum_out=acc[:, pt : pt + 1]
                )
            if "suma" in ops and use_pe:
                junkA = acc_pool.tile([1, W], F32)
                nc.scalar.activation(
                    junkA[:], psumA[:, :], AF.Copy,
                    accum_out=acc[0:1, 2 * T : 2 * T + 1],
                )
            nc.sync.dma_start(out=out_acc, in_=acc[:])
    return nc


def build_nc_v3(n_tiles=4, width=None, io_bufs=3, tmp_bufs=3, use_pe=True,
                ops=V3_ALL_OPS):
    if width is None:
        width = NP_CORE // (n_tiles * P)
    nc = bacc.Bacc("TRN2", target_bir_lowering=False, debug=False)
    conf = nc.dram_tensor("conf", [n_tiles, P, 4 * width], F8, kind="ExternalInput").ap()
    lg = nc.dram_tensor("lg", [n_tiles, P, 2 * width], F8, kind="ExternalInput").ap()
    n_out = 2 * n_tiles + 1 + (0 if use_pe else n_tiles)
    out_acc = nc.dram_tensor("acc", [P, n_out], F32, kind="ExternalOutput").ap()
    emit_loss_kernel_v3(nc, conf, lg, out_acc, n_tiles, width, io_bufs, tmp_bufs,
                        use_pe, ops)
    nc.finalize()
    _pin_act_table_set(nc, 6)
    return nc


def emit_loss_kernel_v4(nc, cbl, ccd, out_acc, n_tiles, width, io_bufs=3, tmp_bufs=3):
    """bf16/fp8 mixed planar variant.

    cbl: DRAM AP [T, 128, 4W] bf16, planes [A|B|L0|L1] (A/B host-permuted
      confusion logits for argmax=0 / argmax=1; L0/L1 prediction logits).
    ccd: DRAM AP [T, 128, 2W] fp8, planes [C|D] (remaining confusion logits).
    out_acc: [128, 2T+1] f32: cols [0,T) per-partition sum(ln s);
      cols [T,2T) per-partition sum((L1>L0)*(B-A)); acc[0, 2T] = sum(A).

    loss*B*N = sum(ln s) - sum(A) - sum(p*(B-A))
    """
    W = width
    T = n_tiles
    with TileContext(nc) as tc:
        with (
            tc.tile_pool(name="io", bufs=io_bufs) as io_pool,
            tc.tile_pool(name="tmp", bufs=tmp_bufs) as tmp_pool,
            tc.tile_pool(name="accp", bufs=1) as acc_pool,
            tc.tile_pool(name="psum", bufs=1, space="PSUM") as psum_pool,
        ):
            acc = acc_pool.tile([P, 2 * T + 1], F32)
            ones = acc_pool.tile([P, 1], BF16)
            nc.vector.memset(ones[:], 1.0)
            nc.vector.memset(acc[:, 2 * T : 2 * T + 1], 0.0)
            psumA = psum_pool.tile([1, W], F32)
            n_chunks = (W + 511) // 512

            prev = None
            for t in range(T):
                cbl_t = io_pool.tile([P, 4 * W], BF16, tag="cbl")
                ccd_t = io_pool.tile([P, 2 * W], F8, tag="ccd")
                nc.sync.dma_start(out=cbl_t[:], in_=cbl[t])
                nc.sync.dma_start(out=ccd_t[:], in_=ccd[t])

                e_t = tmp_pool.tile([P, 4 * W], BF16, tag="e")
                s2 = tmp_pool.tile([P, 2 * W], BF16, tag="s2")
                s = tmp_pool.tile([P, W], BF16, tag="s")
                dz = tmp_pool.tile([P, W], BF16, tag="dz")
                dl = tmp_pool.tile([P, W], BF16, tag="dl")
                junkp = tmp_pool.tile([P, W], BF16, tag="junkp")

                ca = cbl_t[:]
                A = ca[:, 0:W]
                Bp = ca[:, W : 2 * W]
                L0 = ca[:, 2 * W : 3 * W]
                L1 = ca[:, 3 * W : 4 * W]

                # ACT: exp of A,B then (skewed) ln of the previous tile,
                # then exp of C,D — keeps ACT busy while DVE sums s.
                nc.scalar.activation(e_t[:, 0 : 2 * W], ca[:, 0 : 2 * W], AF.Exp)
                if prev is not None:
                    (ps, pjl, pt) = prev
                    nc.scalar.activation(
                        pjl[:], ps[:], AF.Ln, accum_out=acc[:, pt : pt + 1]
                    )
                nc.scalar.activation(e_t[:, 2 * W : 4 * W], ccd_t[:], AF.Exp)

                # DVE
                nc.vector.tensor_tensor(dz[:], Bp, A, AluOpType.subtract)
                nc.vector.tensor_tensor(dl[:], L1, L0, AluOpType.subtract)
                ea = e_t[:]
                nc.vector.tensor_tensor(
                    s2[:], ea[:, 0 : 2 * W], ea[:, 2 * W : 4 * W], AluOpType.add
                )
                nc.vector.tensor_tensor(
                    s[:], s2[:, 0:W], s2[:, W : 2 * W], AluOpType.add
                )
                nc.vector.scalar_tensor_tensor(
                    junkp[:], dl[:], 0.0, dz[:],
                    AluOpType.is_gt, AluOpType.mult,
                    accum_out=acc[:, T + t : T + t + 1],
                )

                # PE: column sums of the A plane accumulated across tiles
                for c in range(n_chunks):
                    lo, hi = c * 512, min((c + 1) * 512, W)
                    nc.tensor.matmul(
                        psumA[:, lo:hi], ones[:], A[:, lo:hi],
                        start=(t == 0), stop=(t == T - 1),
                    )

                junkl = tmp_pool.tile([P, W], BF16, tag="junkl")
                prev = (s, junkl, t)

            (ps, pjl, pt) = prev
            nc.scalar.activation(pjl[:], ps[:], AF.Ln, accum_out=acc[:, pt : pt + 1])
            junkA = acc_pool.tile([1, W], F32)
            nc.scalar.activation(
                junkA[:], psumA[:, :], AF.Copy,
                accum_out=acc[0:1, 2 * T : 2 * T + 1],
            )
            nc.sync.dma_start(out=out_acc, in_=acc[:])
    return nc


def build_nc_v4(n_tiles=4, width=None, io_bufs=3, tmp_bufs=3):
    if width is None:
        width = NP_CORE // (n_tiles * P)
    nc = bacc.Bacc("TRN2", target_bir_lowering=False, debug=False)
    cbl = nc.dram_tensor("cbl", [n_tiles, P, 4 * width], BF16, kind="ExternalInput").ap()
    ccd = nc.dram_tensor("ccd", [n_tiles, P, 2 * width], F8, kind="ExternalInput").ap()
    out_acc = nc.dram_tensor("acc", [P, 2 * n_tiles + 1], F32, kind="ExternalOutput").ap()
    emit_loss_kernel_v4(nc, cbl, ccd, out_acc, n_tiles, width, io_bufs, tmp_bufs)
    nc.finalize()
    _pin_act_table_set(nc, 6)
    return nc


V5_WIDTHS = (512, 1280, 1280, 768, 256)


def emit_loss_kernel_v5(nc, cbl, ccd, out_acc, widths, io_bufs=3, tmp_bufs=3):
    """Ragged-tile variant of v4: small first tile (fast pipeline fill),
    small last tile (short drain). Per-tile split DMAs so exp(A,B) only
    waits for the [A|B] planes. PE chunk sums all land in one [1,256]
    accumulating PSUM region, flushed by DVE.

    cbl: DRAM AP [128, 4*total] bf16, per-tile blocks [A|B|L0|L1] (w wide each)
    ccd: DRAM AP [128, 2*total] fp8, per-tile blocks [C|D]
    out_acc: [128, 2T+1] f32
    """
    T = len(widths)
    Wmax = max(widths)
    with TileContext(nc) as tc:
        with (
            tc.tile_pool(name="io", bufs=io_bufs) as io_pool,
            tc.tile_pool(name="tmp", bufs=tmp_bufs) as tmp_pool,
            tc.tile_pool(name="accp", bufs=1) as acc_pool,
            tc.tile_pool(name="psum", bufs=1, space="PSUM") as psum_pool,
        ):
            acc = acc_pool.tile([P, 2 * T + 1], F32)
            ones = acc_pool.tile([P, 1], BF16)
            nc.vector.memset(ones[:], 1.0)
            psumA = psum_pool.tile([1, 256], F32)

            n_mm = sum(w // 256 for w in widths)
            prev = None
            off = 0
            mm = 0
            for t, W in enumerate(widths):
                cab_t = io_pool.tile([P, 2 * Wmax], BF16, tag="cab")
                cl_t = io_pool.tile([P, 2 * Wmax], BF16, tag="cl")
                ccd_t = io_pool.tile([P, 2 * Wmax], F8, tag="ccd")
                nc.sync.dma_start(out=cab_t[:, 0 : 2 * W],
                                  in_=cbl[:, 4 * off : 4 * off + 2 * W])
                nc.sync.dma_start(out=ccd_t[:, 0 : 2 * W],
                                  in_=ccd[:, 2 * off : 2 * off + 2 * W])
                nc.sync.dma_start(out=cl_t[:, 0 : 2 * W],
                                  in_=cbl[:, 4 * off + 2 * W : 4 * off + 4 * W])

                e_t = tmp_pool.tile([P, 4 * Wmax], BF16, tag="e")
                s2 = tmp_pool.tile([P, 2 * Wmax], BF16, tag="s2")
                s = tmp_pool.tile([P, Wmax], BF16, tag="s")
                dz = tmp_pool.tile([P, Wmax], BF16, tag="dz")
                dl = tmp_pool.tile([P, Wmax], BF16, tag="dl")
                junkp = tmp_pool.tile([P, Wmax], BF16, tag="junkp")

                A = cab_t[:, 0:W]
                Bp = cab_t[:, W : 2 * W]
                L0 = cl_t[:, 0:W]
                L1 = cl_t[:, W : 2 * W]

                nc.scalar.activation(e_t[:, 0 : 2 * W], cab_t[:, 0 : 2 * W], AF.Exp)
                if prev is not None:
                    (ps, pjl, pw, pt) = prev
                    nc.scalar.activation(
                        pjl[:, 0:pw], ps[:, 0:pw], AF.Ln,
                        accum_out=acc[:, pt : pt + 1],
                    )
                nc.scalar.activation(
                    e_t[:, 2 * W : 4 * W], ccd_t[:, 0 : 2 * W], AF.Exp
                )

                nc.vector.tensor_tensor(dz[:, 0:W], Bp, A, AluOpType.subtract)
                nc.vector.tensor_tensor(dl[:, 0:W], L1, L0, AluOpType.subtract)
                nc.vector.tensor_tensor(
                    s2[:, 0 : 2 * W], e_t[:, 0 : 2 * W], e_t[:, 2 * W : 4 * W],
                    AluOpType.add,
                )
                nc.vector.tensor_tensor(
                    s[:, 0:W], s2[:, 0:W], s2[:, W : 2 * W], AluOpType.add
                )
                nc.vector.scalar_tensor_tensor(
                    junkp[:, 0:W], dl[:, 0:W], 0.0, dz[:, 0:W],
                    AluOpType.is_gt, AluOpType.mult,
                    accum_out=acc[:, T + t : T + t + 1],
                )

                for c in range(W // 256):
                    nc.tensor.matmul(
                        psumA[:, :], ones[:], A[:, c * 256 : (c + 1) * 256],
                        start=(mm == 0), stop=(mm == n_mm - 1),
                    )
                    mm += 1

                junkl = tmp_pool.tile([P, Wmax], BF16, tag="junkl")
                prev = (s, junkl, W, t)
                off += W

            (ps, pjl, pw, pt) = prev
            nc.scalar.activation(
                pjl[:, 0:pw], ps[:, 0:pw], AF.Ln, accum_out=acc[:, pt : pt + 1]
            )
            junkA = acc_pool.tile([1, 256], F32)
            nc.scalar.activation(
                junkA[:], psumA[:, :], AF.Copy,
                accum_out=acc[0:1, 2 * T : 2 * T + 1],
            )
            nc.sync.dma_start(out=out_acc, in_=acc[:])
    return nc


V6_WIDTHS = (512, 896, 1024, 1024, 640)


def emit_loss_kernel_v6(nc, conf, lg, out_acc, widths, io_bufs=3, tmp_bufs=3):
    """All-bf16 conf (single exp per tile), fp8 logits, ragged tiles.

    conf: DRAM AP [128, 4*total] bf16, per-tile blocks [A|B|C|D] (w wide each)
    lg:   DRAM AP [128, 2*total] fp8, per-tile blocks [L0|L1]
    out_acc: [128, 2T+1] f32
    """
    T = len(widths)
    Wmax = max(widths)
    with TileContext(nc) as tc:
        with (
            tc.tile_pool(name="io", bufs=io_bufs) as io_pool,
            tc.tile_pool(name="tmp", bufs=tmp_bufs) as tmp_pool,
            tc.tile_pool(name="accp", bufs=1) as acc_pool,
            tc.tile_pool(name="psum", bufs=1, space="PSUM") as psum_pool,
        ):
            acc = acc_pool.tile([P, 2 * T + 1], F32)
            ones = acc_pool.tile([P, 1], BF16)
            nc.vector.memset(ones[:], 1.0)
            psumA = psum_pool.tile([1, 256], F32)

            n_mm = sum((w + 255) // 256 for w in widths)
            prev = None
            off = 0
            mm = 0
            for t, W in enumerate(widths):
                conf_t = io_pool.tile([P, 4 * Wmax], BF16, tag="conf")
                lg_t = io_pool.tile([P, 2 * Wmax], F8, tag="lg")
                nc.sync.dma_start(out=conf_t[:, 0 : 4 * W],
                                  in_=conf[:, 4 * off : 4 * off + 4 * W])
                nc.sync.dma_start(out=lg_t[:, 0 : 2 * W],
                                  in_=lg[:, 2 * off : 2 * off + 2 * W])

                e_t = tmp_pool.tile([P, 4 * Wmax], BF16, tag="e")
                s2 = tmp_pool.tile([P, 2 * Wmax], BF16, tag="s2")
                s = tmp_pool.tile([P, Wmax], BF16, tag="s")
                dz = tmp_pool.tile([P, Wmax], BF16, tag="dz")
                dl = tmp_pool.tile([P, Wmax], BF16, tag="dl")
                junkp = tmp_pool.tile([P, Wmax], BF16, tag="junkp")

                A = conf_t[:, 0:W]
                Bp = conf_t[:, W : 2 * W]

                nc.scalar.activation(e_t[:, 0 : 4 * W], conf_t[:, 0 : 4 * W], AF.Exp)
                if prev is not None:
                    (ps, pjl, pw, pt) = prev
                    nc.scalar.activation(
                        pjl[:, 0:pw], ps[:, 0:pw], AF.Ln,
                        accum_out=acc[:, pt : pt + 1],
                    )

                nc.vector.tensor_tensor(dz[:, 0:W], Bp, A, AluOpType.subtract)
                nc.vector.tensor_tensor(
                    dl[:, 0:W], lg_t[:, W : 2 * W], lg_t[:, 0:W],
                    AluOpType.subtract,
                )
                nc.vector.tensor_tensor(
                    s2[:, 0 : 2 * W], e_t[:, 0 : 2 * W], e_t[:, 2 * W : 4 * W],
                    AluOpType.add,
                )
                nc.vector.tensor_tensor(
                    s[:, 0:W], s2[:, 0:W], s2[:, W : 2 * W], AluOpType.add
                )
                nc.vector.scalar_tensor_tensor(
                    junkp[:, 0:W], dl[:, 0:W], 0.0, dz[:, 0:W],
                    AluOpType.is_gt, AluOpType.mult,
                    accum_out=acc[:, T + t : T + t + 1],
                )

                for c in range((W + 255) // 256):
                    lo = c * 256
                    hi = min(lo + 256, W)
                    nc.tensor.matmul(
                        psumA[:, 0 : hi - lo], ones[:], A[:, lo:hi],
                        start=(mm == 0), stop=(mm == n_mm - 1),
                    )
                    mm += 1

                junkl = tmp_pool.tile([P, Wmax], BF16, tag="junkl")
                prev = (s, junkl, W, t)
                off += W

            junkA = acc_pool.tile([1, 256], F32)
            nc.scalar.activation(
                junkA[:], psumA[:, :], AF.Copy,
                accum_out=acc[0:1, 2 * T : 2 * T + 1],
            )
            (ps, pjl, pw, pt) = prev
            nc.scalar.activation(
                pjl[:, 0:pw], ps[:, 0:pw], AF.Ln, accum_out=acc[:, pt : pt + 1]
            )
            nc.sync.dma_start(out=out_acc, in_=acc[:])
    return nc


V7_WIDTHS = (384, 640, 1024, 1024, 1024)


def emit_loss_kernel_v7(nc, conf, lg, out_acc, widths, io_bufs=3, tmp_bufs=3,
                        pool_dl=False):
    """All-fp8 ragged variant: 6 B/point DMA, split AB/CD exps for early
    pipeline fill, psum flush hoisted before the final ln.

    conf: DRAM AP [128, 4*total] fp8, per-tile blocks [A|B|C|D]
    lg:   DRAM AP [128, 2*total] fp8, per-tile blocks [L0|L1]
    out_acc: [128, 2T+1] f32
    """
    T = len(widths)
    Wmax = max(widths)
    with TileContext(nc) as tc:
        with (
            tc.tile_pool(name="io", bufs=io_bufs) as io_pool,
            tc.tile_pool(name="tmp", bufs=tmp_bufs) as tmp_pool,
            tc.tile_pool(name="accp", bufs=1) as acc_pool,
            tc.tile_pool(name="psum", bufs=1, space="PSUM") as psum_pool,
        ):
            acc = acc_pool.tile([P, 2 * T + 1], F32)
            ones = acc_pool.tile([P, 1], F8)
            nc.vector.memset(ones[:], 1.0)
            psumA = psum_pool.tile([1, 256], F32)

            n_mm = sum((w + 255) // 256 for w in widths)
            prev = None
            off = 0
            mm = 0
            for t, W in enumerate(widths):
                conf_t = io_pool.tile([P, 4 * Wmax], F8, tag="conf")
                lg_t = io_pool.tile([P, 2 * Wmax], F8, tag="lg")
                nc.sync.dma_start(out=conf_t[:, 0 : 2 * W],
                                  in_=conf[:, 4 * off : 4 * off + 2 * W])
                nc.sync.dma_start(out=conf_t[:, 2 * W : 4 * W],
                                  in_=conf[:, 4 * off + 2 * W : 4 * off + 4 * W])
                nc.sync.dma_start(out=lg_t[:, 0 : 2 * W],
                                  in_=lg[:, 2 * off : 2 * off + 2 * W])

                e_t = tmp_pool.tile([P, 4 * Wmax], BF16, tag="e")
                s2 = tmp_pool.tile([P, 2 * Wmax], BF16, tag="s2")
                s = tmp_pool.tile([P, Wmax], BF16, tag="s")
                dz = tmp_pool.tile([P, Wmax], BF16, tag="dz")
                dl = tmp_pool.tile([P, Wmax], BF16, tag="dl")
                junkp = tmp_pool.tile([P, Wmax], BF16, tag="junkp")

                A = conf_t[:, 0:W]
                Bp = conf_t[:, W : 2 * W]

                nc.scalar.activation(e_t[:, 0 : 2 * W], conf_t[:, 0 : 2 * W], AF.Exp)
                if prev is not None:
                    (ps, pjl, pw, pt) = prev
                    nc.scalar.activation(
                        pjl[:, 0:pw], ps[:, 0:pw], AF.Ln,
                        accum_out=acc[:, pt : pt + 1],
                    )
                nc.scalar.activation(
                    e_t[:, 2 * W : 4 * W], conf_t[:, 2 * W : 4 * W], AF.Exp
                )

                # s-chain feeds the ACT ln — schedule it ahead of the
                # non-critical dz/dl/stt work.
                with tc.high_priority(offset=24):
                    nc.vector.tensor_tensor(
                        s2[:, 0 : 2 * W], e_t[:, 0 : 2 * W],
                        e_t[:, 2 * W : 4 * W], AluOpType.add,
                    )
                    nc.vector.tensor_tensor(
                        s[:, 0:W], s2[:, 0:W], s2[:, W : 2 * W], AluOpType.add
                    )
                nc.vector.tensor_tensor(dz[:, 0:W], Bp, A, AluOpType.subtract)
                dl_eng = nc.gpsimd if pool_dl else nc.vector
                dl_eng.tensor_tensor(
                    dl[:, 0:W], lg_t[:, W : 2 * W], lg_t[:, 0:W],
                    AluOpType.subtract,
                )
                nc.vector.scalar_tensor_tensor(
                    junkp[:, 0:W], dl[:, 0:W], 0.0, dz[:, 0:W],
                    AluOpType.is_gt, AluOpType.mult,
                    accum_out=acc[:, T + t : T + t + 1],
                )

                for c in range((W + 255) // 256):
                    lo = c * 256
                    hi = min(lo + 256, W)
                    nc.tensor.matmul(
                        psumA[:, 0 : hi - lo], ones[:], A[:, lo:hi],
                        start=(mm == 0), stop=(mm == n_mm - 1),
                    )
                    mm += 1

                junkl = tmp_pool.tile([P, Wmax], BF16, tag="junkl")
                prev = (s, junkl, W, t)
                off += W

            junkA = acc_pool.tile([1, 256], F32)
            nc.scalar.activation(
                junkA[:], psumA[:, :], AF.Copy,
                accum_out=acc[0:1, 2 * T : 2 * T + 1],
            )
            (ps, pjl, pw, pt) = prev
            nc.scalar.activation(
                pjl[:, 0:pw], ps[:, 0:pw], AF.Ln, accum_out=acc[:, pt : pt + 1]
            )
            nc.sync.dma_start(out=out_acc, in_=acc[:])
    return nc


def build_nc_v7(widths=V7_WIDTHS, io_bufs=3, tmp_bufs=3, pool_dl=False):
    total = sum(widths)
    assert total * P == NP_CORE
    nc = bacc.Bacc("TRN2", target_bir_lowering=False, debug=False)
    conf = nc.dram_tensor("conf", [P, 4 * total], F8, kind="ExternalInput").ap()
    lg = nc.dram_tensor("lg", [P, 2 * total], F8, kind="ExternalInput").ap()
    T = len(widths)
    out_acc = nc.dram_tensor("acc", [P, 2 * T + 1], F32, kind="ExternalOutput").ap()
    emit_loss_kernel_v7(nc, conf, lg, out_acc, widths, io_bufs, tmp_bufs, pool_dl)
    nc.finalize()
    _pin_act_table_set(nc, 6)
    return nc


def shard_inputs_v7(pred_logits, pred_confusion, target_classes, widths=V7_WIDTHS):
    import ml_dtypes

    total = sum(widths)
    f8 = ml_dtypes.float8_e4m3
    cf = np.asarray(pred_confusion, dtype=np.float32)
    lgf = np.asarray(pred_logits, dtype=np.float32)
    g = np.asarray(target_classes)
    in_maps = []
    for i in range(M):
        sl = slice(i * BS, (i + 1) * BS)
        ci, gi = cf[sl], g[sl]
        planes = np.where(
            (gi == 1)[..., None], ci[..., _ORD_G1], ci[..., _ORD_G0]
        ).reshape(P, total, 4)
        lgi = lgf[sl].reshape(P, total, 2)
        conf = np.empty((P, 4 * total), dtype=f8)
        lg = np.empty((P, 2 * total), dtype=f8)
        off = 0
        for w in widths:
            blk = planes[:, off : off + w]
            lblk = lgi[:, off : off + w]
            base = 4 * off
            for k in range(4):
                conf[:, base + k * w : base + (k + 1) * w] = blk[..., k]
            cbase = 2 * off
            lg[:, cbase : cbase + w] = lblk[..., 0]
            lg[:, cbase + w : cbase + 2 * w] = lblk[..., 1]
            off += w
        in_maps.append({"conf": conf, "lg": lg})
    return in_maps


def kernel_v7(pred_logits, pred_confusion, target_classes, widths=V7_WIDTHS,
              pool_dl=False):
    key = ("nc7", widths, pool_dl)
    if key not in _CACHED:
        _CACHED[key] = build_nc_v7(widths=widths, pool_dl=pool_dl)
    in_maps = shard_inputs_v7(pred_logits, pred_confusion, target_classes,
                              widths=widths)
    results = run_bass_kernel_spmd(_CACHED[key], in_maps, list(range(M))).results
    return reduce_v3(results, n_tiles=len(widths))


def build_nc_v6(widths=V6_WIDTHS, io_bufs=3, tmp_bufs=3):
    total = sum(widths)
    assert total * P == NP_CORE
    nc = bacc.Bacc("TRN2", target_bir_lowering=False, debug=False)
    conf = nc.dram_tensor("conf", [P, 4 * total], BF16, kind="ExternalInput").ap()
    lg = nc.dram_tensor("lg", [P, 2 * total], F8, kind="ExternalInput").ap()
    T = len(widths)
    out_acc = nc.dram_tensor("acc", [P, 2 * T + 1], F32, kind="ExternalOutput").ap()
    emit_loss_kernel_v6(nc, conf, lg, out_acc, widths, io_bufs, tmp_bufs)
    nc.finalize()
    _pin_act_table_set(nc, 6)
    return nc


def shard_inputs_v6(pred_logits, pred_confusion, target_classes, widths=V6_WIDTHS):
    import ml_dtypes

    total = sum(widths)
    f8 = ml_dtypes.float8_e4m3
    bf = ml_dtypes.bfloat16
    cf = np.asarray(pred_confusion, dtype=np.float32)
    lgf = np.asarray(pred_logits, dtype=np.float32)
    g = np.asarray(target_classes)
    in_maps = []
    for i in range(M):
        sl = slice(i * BS, (i + 1) * BS)
        ci, gi = cf[sl], g[sl]
        planes = np.where(
            (gi == 1)[..., None], ci[..., _ORD_G1], ci[..., _ORD_G0]
        ).reshape(P, total, 4)
        lgi = lgf[sl].reshape(P, total, 2)
        conf = np.empty((P, 4 * total), dtype=bf)
        lg = np.empty((P, 2 * total), dtype=f8)
        off = 0
        for w in widths:
            blk = planes[:, off : off + w]
            lblk = lgi[:, off : off + w]
            base = 4 * off
            for k in range(4):
                conf[:, base + k * w : base + (k + 1) * w] = blk[..., k]
            cbase = 2 * off
            lg[:, cbase : cbase + w] = lblk[..., 0]
            lg[:, cbase + w : cbase + 2 * w] = lblk[..., 1]
            off += w
        in_maps.append({"conf": conf, "lg": lg})
    return in_maps


def kernel_v6(pred_logits, pred_confusion, target_classes, widths=V6_WIDTHS):
    key = ("nc6", widths)
    if key not in _CACHED:
        _CACHED[key] = build_nc_v6(widths=widths)
    in_maps = shard_inputs_v6(pred_logits, pred_confusion, target_classes,
                              widths=widths)
    results = run_bass_kernel_spmd(_CACHED[key], in_maps, list(range(M))).results
    return reduce_v3(results, n_tiles=len(widths))


def build_nc_v5(widths=V5_WIDTHS, io_bufs=3, tmp_bufs=3):
    total = sum(widths)
    assert total * P == NP_CORE
    nc = bacc.Bacc("TRN2", target_bir_lowering=False, debug=False)
    cbl = nc.dram_tensor("cbl", [P, 4 * total], BF16, kind="ExternalInput").ap()
    ccd = nc.dram_tensor("ccd", [P, 2 * total], F8, kind="ExternalInput").ap()
    T = len(widths)
    out_acc = nc.dram_tensor("acc", [P, 2 * T + 1], F32, kind="ExternalOutput").ap()
    emit_loss_kernel_v5(nc, cbl, ccd, out_acc, widths, io_bufs, tmp_bufs)
    nc.finalize()
    _pin_act_table_set(nc, 6)
    return nc


def shard_inputs_v5(pred_logits, pred_confusion, target_classes, widths=V5_WIDTHS):
    import ml_dtypes

    total = sum(widths)
    f8 = ml_dtypes.float8_e4m3
    bf = ml_dtypes.bfloat16
    cf = np.asarray(pred_confusion, dtype=np.float32)
    lgf = np.asarray(pred_logits, dtype=np.float32)
    g = np.asarray(target_classes)
    in_maps = []
    for i in range(M):
        sl = slice(i * BS, (i + 1) * BS)
        ci, gi = cf[sl], g[sl]
        planes = np.where(
            (gi == 1)[..., None], ci[..., _ORD_G1], ci[..., _ORD_G0]
        ).reshape(P, total, 4)
        lgi = lgf[sl].reshape(P, total, 2)
        cbl = np.empty((P, 4 * total), dtype=bf)
        ccd = np.empty((P, 2 * total), dtype=f8)
        off = 0
        for w in widths:
            blk = planes[:, off : off + w]      # [P, w, 4]
            lblk = lgi[:, off : off + w]        # [P, w, 2]
            base = 4 * off
            cbl[:, base : base + w] = blk[..., 0]
            cbl[:, base + w : base + 2 * w] = blk[..., 1]
            cbl[:, base + 2 * w : base + 3 * w] = lblk[..., 0]
            cbl[:, base + 3 * w : base + 4 * w] = lblk[..., 1]
            cbase = 2 * off
            ccd[:, cbase : cbase + w] = blk[..., 2]
            ccd[:, cbase + w : cbase + 2 * w] = blk[..., 3]
            off += w
        in_maps.append({"cbl": cbl, "ccd": ccd})
    return in_maps


def kernel_v5(pred_logits, pred_confusion, target_classes, widths=V5_WIDTHS):
    key = ("nc5", widths)
    if key not in _CACHED:
        _CACHED[key] = build_nc_v5(widths=widths)
    in_maps = shard_inputs_v5(pred_logits, pred_confusion, target_classes,
                              widths=widths)
    results = run_bass_kernel_spmd(_CACHED[key], in_maps, list(range(M))).results
    return reduce_v3(results, n_tiles=len(widths))


def shard_inputs_v4(pred_logits, pred_confusion, target_classes, n_tiles=4, width=None):
    import ml_dtypes

    if width is None:
        width = NP_CORE // (n_tiles * P)
    T_, W_ = n_tiles, width
    f8 = ml_dtypes.float8_e4m3
    bf = ml_dtypes.bfloat16
    cf = np.asarray(pred_confusion, dtype=np.float32)
    lgf = np.asarray(pred_logits, dtype=np.float32)
    g = np.asarray(target_classes)
    in_maps = []
    for i in range(M):
        sl = slice(i * BS, (i + 1) * BS)
        ci, gi = cf[sl], g[sl]
        planes = np.where(
            (gi == 1)[..., None], ci[..., _ORD_G1], ci[..., _ORD_G0]
        ).reshape(T_, P, W_, 4)
        lgi = lgf[sl].reshape(T_, P, W_, 2)
        cbl = np.empty((T_, P, 4, W_), dtype=bf)
        cbl[:, :, 0] = planes[..., 0]
        cbl[:, :, 1] = planes[..., 1]
        cbl[:, :, 2] = lgi[..., 0]
        cbl[:, :, 3] = lgi[..., 1]
        ccd = np.empty((T_, P, 2, W_), dtype=f8)
        ccd[:, :, 0] = planes[..., 2]
        ccd[:, :, 1] = planes[..., 3]
        in_maps.append({"cbl": cbl.reshape(T_, P, 4 * W_),
                        "ccd": ccd.reshape(T_, P, 2 * W_)})
    return in_maps


def kernel_v4(pred_logits, pred_confusion, target_classes, n_tiles=4):
    key = ("nc4", n_tiles)
    if key not in _CACHED:
        _CACHED[key] = build_nc_v4(n_tiles=n_tiles)
    in_maps = shard_inputs_v4(pred_logits, pred_confusion, target_classes,
                              n_tiles=n_tiles)
    results = run_bass_kernel_spmd(_CACHED[key], in_maps, list(range(M))).results
    return reduce_v3(results, n_tiles=n_tiles)


_ORD_G1 = [2, 1, 0, 3]
_ORD_G0 = [0, 3, 2, 1]


def shard_inputs_v3(pred_logits, pred_confusion, target_classes, n_tiles=4, width=None):
    import ml_dtypes

    if width is None:
        width = NP_CORE // (n_tiles * P)
    T_, W_ = n_tiles, width
    f8 = ml_dtypes.float8_e4m3
    cf = np.asarray(pred_confusion, dtype=np.float32)
    lgf = np.asarray(pred_logits, dtype=np.float32)
    g = np.asarray(target_classes)
    in_maps = []
    for i in range(M):
        sl = slice(i * BS, (i + 1) * BS)
        ci, gi = cf[sl], g[sl]
        planes = np.where(
            (gi == 1)[..., None], ci[..., _ORD_G1], ci[..., _ORD_G0]
        )
        conf = (
            planes.reshape(T_, P, W_, 4).transpose(0, 1, 3, 2).astype(f8)
        ).reshape(T_, P, 4 * W_)
        lg = (
            lgf[sl].reshape(T_, P, W_, 2).transpose(0, 1, 3, 2).astype(f8)
        ).reshape(T_, P, 2 * W_)
        in_maps.append({"conf": np.ascontiguousarray(conf),
                        "lg": np.ascontiguousarray(lg)})
    return in_maps


def reduce_v3(results, n_tiles=4):
    T_ = n_tiles
    total = 0.0
    for r in results:
        a = np.asarray(r["acc"], dtype=np.float64)
        total += a[:, :T_].sum() - a[:, T_ : 2 * T_].sum() - a[0, 2 * T_]
        if a.shape[1] > 2 * T_ + 1:  # use_pe=False: per-tile DVE sums of A
            total -= a[:, 2 * T_ + 1 :].sum()
    return np.float32(total / (B * N))


def kernel_v3(pred_logits, pred_confusion, target_classes):
    if "nc3" not in _CACHED:
        _CACHED["nc3"] = build_nc_v3()
    in_maps = shard_inputs_v3(pred_logits, pred_confusion, target_classes)
    results = run_bass_kernel_spmd(_CACHED["nc3"], in_maps, list(range(M))).results
    return reduce_v3(results)


_CACHED = {}


def _get_nc():
    if "nc" not in _CACHED:
        _CACHED["nc"] = build_nc()
    return _CACHED["nc"]


def kernel(pred_logits, pred_confusion, target_classes):
    nc = _get_nc()
    in_maps = shard_inputs(pred_logits, pred_confusion, target_classes)
    results = run_bass_kernel_spmd(nc, in_maps, list(range(M))).results
    total = 0.0
    for r in results:
        a = np.asarray(r["acc"], dtype=np.float64)
        total += a[:, :T].sum() - a[:, T:].sum()
    return np.float32(total / (B * N))


def reduce_v2(results):
    total = 0.0
    for r in results:
        a = np.asarray(r["acc"], dtype=np.float64)
        total += a[:, :T].sum() - a[0, T] - a[:, T + 1 :].sum()
    return np.float32(total / (B * N))


def kernel_v2(pred_logits, pred_confusion, target_classes):
    if "nc2" not in _CACHED:
        _CACHED["nc2"] = build_nc_v2()
    in_maps = shard_inputs_v2(pred_logits, pred_confusion, target_classes)
    results = run_bass_kernel_spmd(_CACHED["nc2"], in_maps, list(range(M))).results
    return reduce_v2(results)

